# revision 18
# baseline (speedup 1.0000x reference)
"""Trainium2 Bass kernel for nn_CONV_tiny_add_partial_558345748883.

Network: 3x [conv5x5(pad2) -> BN -> avgpool2 -> clip01] -> conv4x4(valid) -> BN1d
Input x_in [1024, 3, 32, 32] f32; output [1024, 10] f32.

v3 strategy
-----------
Measured law: per-MATMUL cost ~34ns (sem-completion serialization) once >=7
subtiles are concurrent; per-tile stream 1 col/cycle; so minimize MM count by
maximizing N*M per MM (N<=512, M = output width).

- K-packing via phase-split activation layouts (zero-copy):
  L1: x columns phase-split host-side -> K=36 (2 dx-phases x 6dy x 3ci),
      3 taps, 8 tiles (2 row-groups x 4 col bands), M=32, N=512.
  a2/a3 stored Y-PARITY-SPLIT: partitions [lane(2) x parity(2) x ci(32)].
  The L1/L2 MMs write psum col bands = (dst-lane, out-y-parity), so the
  PSUM->SBUF evac is identity on all 128 partitions, and the next layer
  gets K=64 = (parity, ci) with tap PAIRS at a uniform free-dim offset:
  L2: 18 passes (6 dx x 3 dy-pairs), 8 tiles 64x32, N=512.
  L3: 18 passes, 4 tiles 64x64 (M=64 native), N=512.
  L4: stag [parity x 64ci]; 16 taps x 1 MM (N=128), rows alternate.
- fp8e4 for L1 input + wl1 (halves input DMA; rel err ~1.1e-2 < 2e-2).
- Input DMA: 8 chunks on sync/scalar queues only, sg0+1 first, weights after.
- Evac: Relu(x+beta) split ACT/DVE; upper clip min(.,1) strided DVE passes.
"""
import os
import sys
import numpy as np

for _p in ("/opt/trn_rl_repo", "/root/.axon_site/_ro/trn_rl_repo"):
    if os.path.isdir(_p) and _p not in sys.path:
        sys.path.append(_p)

import concourse.bass as bass
import concourse.bacc as bacc
import concourse.mybir as mybir
from concourse.tile import TileContext

EPS = 1e-5
N_CORES = 8
DT = mybir.dt.float16
FP8 = mybir.dt.float8e4
F32 = mybir.dt.float32
AF = mybir.ActivationFunctionType

S = 128


# ----------------------------------------------------------------------------
# Host-side prep
# ----------------------------------------------------------------------------

def _fold_w(w, g, b, m, v):
    inv = g / np.sqrt(v + EPS)
    Wp = np.zeros((w.shape[0], w.shape[1], 6, 6), np.float32)
    for r in (0, 1):
        for s_ in (0, 1):
            Wp[:, :, r:r + 5, s_:s_ + 5] += w
    Wp *= 0.25 * inv[:, None, None, None]
    beta = (b - m * inv).astype(np.float32)
    return Wp.astype(np.float32), beta


def host_prep_weights(inputs):
    W1, beta1 = _fold_w(inputs['w1'], inputs['g1'], inputs['b1'], inputs['m1'], inputs['v1'])
    W2, beta2 = _fold_w(inputs['w2'], inputs['g2'], inputs['b2'], inputs['m2'], inputs['v2'])
    W3, beta3 = _fold_w(inputs['w3'], inputs['g3'], inputs['b3'], inputs['m3'], inputs['v3'])
    inv4 = inputs['g4'] / np.sqrt(inputs['v4'] + EPS)
    beta4 = (inputs['b4'] - inputs['m4'] * inv4).astype(np.float32)
    W4 = (inputs['w4'] * inv4[:, None, None, None]).astype(np.float32)

    d = {}
    # L1 lhsT per tap fj (dx = 2*fj + xp): K=36 rows (xp, dy, ci), M=32 co.
    wl1 = np.zeros((36, 3 * 32), np.float32)
    for fj in range(3):
        for xp in range(2):
            blk = W1[:, :, :, 2 * fj + xp].transpose(2, 1, 0).reshape(18, 32)
            wl1[18 * xp:18 * xp + 18, 32 * fj:32 * fj + 32] = blk
    w1r = np.zeros((128, 96), np.float32)
    w1r[0:36] = wl1
    w1r[64:100] = wl1
    d['wl1'] = w1r.astype(mybir.dt.np(FP8))
    # L2 lhsT per pass t2 = j*6+f (dy pair e = 2j+q): K=64 rows (q, ci), M=32.
    wl2 = np.zeros((64, 18 * 32), np.float32)
    for j in range(3):
        for f in range(6):
            t2 = j * 6 + f
            for q in range(2):
                wl2[32 * q:32 * q + 32, 32 * t2:32 * t2 + 32] = \
                    W2[:, :, 2 * j + q, f].T
    w2r = np.zeros((128, 576), np.float32)
    w2r[0:64] = wl2
    w2r[64:128] = wl2
    d['wl2'] = w2r.astype(np.float16)
    # L3 lhsT per pass: K=64 (q, ci), M=64 co.
    wl3 = np.zeros((64, 18 * 64), np.float32)
    for j in range(3):
        for f in range(6):
            t2 = j * 6 + f
            for q in range(2):
                wl3[32 * q:32 * q + 32, 64 * t2:64 * t2 + 64] = \
                    W3[:, :, 2 * j + q, f].T
    w3r = np.zeros((128, 1152), np.float32)
    w3r[0:64] = wl3
    w3r[64:128] = wl3
    d['wl3'] = w3r.astype(np.float16)
    # L4 lhsT per tap t = u*4+v: K=64 ci, M=10; row half = u parity.
    wl4 = W4.transpose(1, 2, 3, 0).reshape(64, 16 * 10)
    w4r = np.zeros((128, 160), np.float32)
    w4r[0:64] = wl4
    w4r[64:128] = wl4
    d['wl4'] = w4r.astype(np.float16)

    # bias columns:
    # 0: beta1 (ACT relu evac, class-0)      1: -beta1 (class-1 lower clip)
    # 2: 1-beta1 (class-1 upper clip)        3: beta2 (L2 psum, class-0)
    # 4: beta2 + C2 (L2 psum, class-1)       5: beta3    6: beta4
    C2 = (W2.sum(axis=(2, 3)) @ beta1).astype(np.float32)  # [co]
    bt = np.zeros((128, 8), np.float32)
    bt[:, 0] = np.tile(beta1, 4)
    bt[:, 1] = np.tile(-beta1, 4)
    bt[:, 2] = np.tile(1.0 - beta1, 4)
    bt[:, 3] = np.tile(beta2, 4)
    bt[:, 4] = np.tile(beta2 + C2, 4)
    bt[:, 5] = np.tile(beta3, 2)
    bt[0:10, 6] = beta4
    d['betas'] = bt
    return d


def host_prep_x(x_core):
    """[128, 3, 32, 32] -> x_l1 [2 R, 36, 4 cp, 4608] fp8.

    Partition k = 18*xp + 3*dy + ci (K=36).  Free, per chunk-pair cp
    (2 sgs): [sg2, j2, y16, xh18, sf4].
    x_l1[R, k, cp, ...] = xpad[n, ci, 2y+dy, 2xh+xp],
      n = ((sg*2 + R)*2 + j)*4 + sf,  sg = 2*cp + sg2.
    """
    xp_ = np.zeros((128, 3, 36, 36), np.float32)
    xp_[:, :, 2:34, 2:34] = x_core
    # [n, dy, ci, y, x] stride-2 rows
    arr = np.stack([xp_[:, :, dy:dy + 32:2, :] for dy in range(6)], axis=1)
    # x phase split -> [n, xp, dy, ci, y, xh]
    arr = arr.reshape(128, 6, 3, 16, 18, 2).transpose(0, 5, 1, 2, 3, 4)
    arr = arr.reshape(128, 36, 16, 18)          # [n, k, y, xh]
    # n = ((sg*2+R)*2+j)*4+sf -> [sg8, R2, j2, sf4]
    arr = arr.reshape(8, 2, 2, 4, 36, 16, 18)   # [sg, R, j, sf, k, y, xh]
    arr = arr.reshape(4, 2, 2, 2, 4, 36, 16, 18)  # [cp, sg2, R, j, sf, k, y, xh]
    out = arr.transpose(2, 5, 0, 1, 3, 6, 7, 4)   # [R, k, cp, sg2, j, y, xh, sf]
    out = out.reshape(2, 36, 4, 4608)
    return np.ascontiguousarray(out).astype(mybir.dt.np(FP8))


# ----------------------------------------------------------------------------
# Bass program
# ----------------------------------------------------------------------------

def build_program():
    nc = bacc.Bacc(target_bir_lowering=False)

    x_l1 = nc.dram_tensor("x_l1", [2, 36, 4, 4608], FP8, kind="ExternalInput")
    wl1 = nc.dram_tensor("wl1", [128, 96], FP8, kind="ExternalInput")
    wl2 = nc.dram_tensor("wl2", [128, 576], DT, kind="ExternalInput")
    wl3 = nc.dram_tensor("wl3", [128, 1152], DT, kind="ExternalInput")
    wl4 = nc.dram_tensor("wl4", [128, 160], DT, kind="ExternalInput")
    betas = nc.dram_tensor("betas", [128, 8], F32, kind="ExternalInput")
    y = nc.dram_tensor("y", [10, 128], F32, kind="ExternalOutput")

    with TileContext(nc) as tc:
        with tc.tile_pool(name="consts", bufs=1) as cpool:
            wl1_t = cpool.tile([128, 96], FP8, name="wl1_t")
            wl2_t = cpool.tile([128, 576], DT, name="wl2_t")
            wl3_t = cpool.tile([128, 1152], DT, name="wl3_t")
            wl4_t = cpool.tile([128, 160], DT, name="wl4_t")
            betas_t = cpool.tile([128, 8], F32, name="betas_t")
            scr = cpool.tile([128, 512], DT, name="scr")
            nc.sync.dma_start(wl1_t[:, :], wl1.ap())
            nc.sync.dma_start(betas_t[:, :], betas.ap())
            nc.scalar.dma_start(wl2_t[:, :], wl2.ap())
            nc.scalar.dma_start(wl3_t[:, :], wl3.ap())
            nc.scalar.dma_start(wl4_t[:, :], wl4.ap())
            nc.vector.memset(scr[:, :], 0.0)

            def flush_weights():
                pass

            # a2: [lane2 x par2 x ci32][yh10, x20, s64]; a3: [yh6, x12, s64]
            a2 = cpool.tile([128, 10 * 20 * 64], DT, name="a2")
            a3 = cpool.tile([128, 6 * 12 * 64], DT, name="a3")
            stag = cpool.tile([128, 8 * 128], DT, name="stag")  # [par x ci][px8, s128]
            out_sb = cpool.tile([128, 128], F32, name="out_sb")

            a2v = a2.rearrange("p (y x s) -> p y x s", y=10, x=20)
            a3v = a3.rearrange("p (y x s) -> p y x s", y=6, x=12)
            stagv = stag.rearrange("p (t s) -> p t s", t=8)

            def memset_borders(t_, YH, W_, ns):
                # phase-plane pad: first+last yh row, and 1 col each side
                v = t_[:, :]
                nc.vector.memset(
                    bass.AP(v.tensor, v.offset,
                            [v.ap[0], [(YH - 1) * W_ * ns, 2], [1, W_ * ns]]), 0.0)
                nc.vector.memset(
                    bass.AP(v.tensor, v.offset + W_ * ns,
                            [v.ap[0], [W_ * ns, YH - 2], [(W_ - 2) * ns, 2], [1, 2 * ns]]),
                    0.0)

            memset_borders(a2, 10, 20, 64)
            memset_borders(a3, 6, 12, 64)

            # class-1 (a' = a - beta1) border value is -beta1, slots 32..63
            av = a2[:, :]
            nc.vector.tensor_scalar_add(
                bass.AP(av.tensor, av.offset + 32,
                        [av.ap[0], [9 * 20 * 64, 2], [64, 20], [1, 32]]),
                bass.AP(av.tensor, av.offset + 32,
                        [av.ap[0], [9 * 20 * 64, 2], [64, 20], [1, 32]]),
                betas_t[:, 1:2])
            for xoff in (0, 18 * 64):
                bap = bass.AP(av.tensor, av.offset + 20 * 64 + xoff + 32,
                              [av.ap[0], [20 * 64, 8], [64, 2], [1, 32]])
                nc.vector.tensor_scalar_add(bap, bap, betas_t[:, 1:2])

            def evac(engine_is_act, dst, src_ap, bias_ap):
                if engine_is_act:
                    nc.scalar.activation(dst, src_ap, AF.Relu,
                                         bias=bias_ap, scale=1.0)
                else:
                    nc.vector.tensor_scalar(
                        dst, src_ap, bias_ap, 0.0,
                        mybir.AluOpType.add, mybir.AluOpType.max)

            with (
                tc.tile_pool(name="l1io", bufs=3) as l1pool,
                tc.tile_pool(name="ps", bufs=8, space="PSUM") as pspool,
            ):
                # ---- HAM warmup: dummy matmuls fill the input-DMA window ----
                wps = pspool.tile([128, 512], F32, name="wps", tag="ps")
                for wi in range(8):
                    nc.tensor.matmul(
                        wps[0:32, :], scr[0:32, 0:32], scr[0:32, 0:512],
                        start=True, stop=True, skip_group_check=True,
                        tile_position=(0, 0),
                    )
                nc.vector.tensor_scalar_min(scr[0:1, 0:1], wps[0:1, 0:1], 1e30)

                # ================= L1 =================
                # chunk-pair cp covers sgs {2cp, 2cp+1}
                for cp in range(4):
                    xt = l1pool.tile([128, 4608], FP8, name="xt", tag="xt")
                    xv = xt.rearrange("p (g j y xh s) -> p g j y xh s",
                                      g=2, j=2, y=16, xh=18)
                    for R in range(2):
                        nc.sync.dma_start(xv[64 * R:64 * R + 36],
                                          x_l1.ap()[R, :, cp])
                    for g in range(2):
                        sg = 2 * cp + g
                        pl1 = [pspool.tile([128, 512], F32, name=f"ps1_{R}", tag="ps")
                               for R in range(2)]
                        for fj in range(3):
                            for R in range(2):
                                lhsT = wl1_t[64 * R:64 * R + 36,
                                             32 * fj:32 * fj + 32]
                                for j in range(2):
                                    for p in range(2):
                                        rhs = xv[64 * R:64 * R + 36, g, j,
                                                 p:16:2, fj:fj + 16, :]
                                        nc.tensor.matmul(
                                            pl1[R][64 * j + 32 * p:64 * j + 32 * p + 32, :],
                                            lhsT, rhs,
                                            start=(fj == 0), stop=(fj == 2),
                                            skip_group_check=True,
                                            tile_position=(64 * R, 64 * j + 32 * p),
                                        )
                        # psum [128 = (j,p) x 32co][y8, x16, sf4]
                        # slots: s1 = 32*R + 4*sg + sf
                        # R=0 (class 0): ACT relu(z+b1) then DVE/Pool min-clip
                        # R=1 (class 1): one DVE clip(z, -b1, 1-b1)
                        src0 = pl1[0][:, :].rearrange(
                            "p (y x s) -> p y x s", y=8, x=16)
                        dst0 = a2v[:, 1:9, 2:18, 4 * sg:4 * sg + 4]
                        nc.scalar.activation(dst0, src0, AF.Relu,
                                             bias=betas_t[:, 0:1], scale=1.0)
                        src1 = pl1[1][:, :].rearrange(
                            "p (y x s) -> p y x s", y=8, x=16)
                        dst1 = a2v[:, 1:9, 2:18, 32 + 4 * sg:32 + 4 * sg + 4]
                        nc.vector.tensor_scalar(
                            dst1, src1, betas_t[:, 1:2], betas_t[:, 2:3],
                            mybir.AluOpType.max, mybir.AluOpType.min)
                        clip_eng = nc.vector if sg % 2 == 0 else nc.gpsimd
                        clip_eng.tensor_scalar_min(
                            dst0, dst0, 1.0)

                # ================= L2 =================
                # passes t2 = j*6+f: K=64 (parity q, ci); out-y parity p' banded
                pl2 = [pspool.tile([128, 512], F32, name=f"ps2_{k}", tag="ps")
                       for k in range(4)]  # k = 2*L + r2
                for t2 in range(18):
                    j, f = t2 // 6, t2 % 6
                    for L in range(2):
                        lhsT = wl2_t[64 * L:64 * L + 64, 32 * t2:32 * t2 + 32]
                        for r2 in range(2):
                            for jp in range(2):  # dst lane j'
                                for p in range(2):  # out-y parity
                                    rhs = a2v[64 * L:64 * L + 64,
                                              j + p:j + p + 7:2, f:f + 15:2,
                                              32 * r2 + 16 * jp:32 * r2 + 16 * jp + 16]
                                    nc.tensor.matmul(
                                        pl2[2 * L + r2][64 * jp + 32 * p:
                                                        64 * jp + 32 * p + 32, :],
                                        lhsT, rhs,
                                        start=(t2 == 0), stop=(t2 == 17),
                                        skip_group_check=True,
                                        tile_position=(64 * L, 64 * jp + 32 * p),
                                    )
                for k in range(4):
                    L, r2 = k // 2, k % 2
                    src = pl2[k][:, :].rearrange(
                        "p (y x s) -> p y x s", y=4, x=8)
                    sl = 16 * (2 * L + r2)
                    dst = a3v[:, 1:5, 2:10, sl:sl + 16]
                    # bias col 3 (class 0) or 4 (class 1 = r2) includes the
                    # a'-representation correction C2
                    evac(k % 2 == 0, dst, src, betas_t[:, 3 + r2:4 + r2])
                    clip_eng = nc.vector if k % 2 == 0 else nc.gpsimd
                    clip_eng.tensor_scalar_min(dst, dst, 1.0)

                # ================= L3 =================
                # 4 tiles: rows = a3 lane L', cols = out-y parity band p''
                pl3 = [pspool.tile([128, 512], F32, name=f"ps3_{k}", tag="ps")
                       for k in range(2)]  # k = L'
                for t2 in range(18):
                    j, f = t2 // 6, t2 % 6
                    for Lp in range(2):
                        lhsT = wl3_t[64 * Lp:64 * Lp + 64, 64 * t2:64 * t2 + 64]
                        for p in range(2):
                            rhs = a3v[64 * Lp:64 * Lp + 64,
                                      j + p:j + p + 3:2, f:f + 7:2, :]
                            nc.tensor.matmul(
                                pl3[Lp][64 * p:64 * p + 64, :], lhsT, rhs,
                                start=(t2 == 0), stop=(t2 == 17),
                                skip_group_check=True,
                                tile_position=(64 * Lp, 64 * p),
                            )
                # evac: psum [128 = par x 64co][y2, x4, s64] -> stag px = y*4+x
                for Lp in range(2):
                    src = pl3[Lp][:, :].rearrange(
                        "p (y x s) -> p (y x) s", y=2, x=4)
                    dst = stagv[:, :, 64 * Lp:64 * Lp + 64]
                    evac(Lp == 0, dst, src, betas_t[:, 5:6])
                    nc.vector.tensor_scalar_min(dst, dst, 1.0)

                # ================= L4 =================
                # separate psum per row-parity stream (avoid concurrent
                # accumulation races into one region), then add at evac
                ps4 = [pspool.tile([128, 128], F32, name=f"ps4_{q}", tag="ps")
                       for q in range(2)]
                nseen = [0, 0]
                for t in range(16):
                    u, v = t // 4, t % 4
                    q = u % 2
                    lhsT = wl4_t[64 * q:64 * q + 64, 10 * t:10 * t + 10]
                    rhs = stagv[64 * q:64 * q + 64, (u // 2) * 4 + v, :]
                    nc.tensor.matmul(
                        ps4[q][0:10, :], lhsT, rhs,
                        start=(nseen[q] == 0), stop=(nseen[q] == 7),
                        skip_group_check=True,
                        tile_position=(64 * q, 0),
                    )
                    nseen[q] += 1
                nc.scalar.activation(
                    out_sb[0:10, :], ps4[0][0:10, :],
                    AF.Identity, bias=betas_t[0:10, 6:7], scale=1.0,
                )
                nc.vector.tensor_tensor(
                    out_sb[0:10, :], ps4[1][0:10, :], out_sb[0:10, :],
                    mybir.AluOpType.add)
                nc.sync.dma_start(y.ap(), out_sb[0:10, :])

        return nc


_NC_CACHE = None


def get_program():
    global _NC_CACHE
    if _NC_CACHE is None:
        nc = build_program()
        if not nc.is_finalized():
            nc.finalize()
        _NC_CACHE = nc
    return _NC_CACHE


def make_in_maps(inputs, n_cores=N_CORES):
    wdict = host_prep_weights(inputs)
    in_maps = []
    for c in range(n_cores):
        x_core = np.asarray(inputs['x_in'][c * S:(c + 1) * S], np.float32)
        m = {'x_l1': host_prep_x(x_core)}
        m.update(wdict)
        in_maps.append(m)
    return in_maps


def _core_sample(col):
    """Output column (0..127) -> per-core sample index n."""
    Lp, t = col // 64, col % 64          # stag: s3 = 64*L' + slot2
    half, k = t // 16, t % 16            # slot2 = 16*(2L + r2) + k
    L, r2 = half // 2, half % 2
    jp = Lp                              # dst a3 lane = j'
    s1 = 32 * r2 + 16 * jp + k           # a2 slot of lane L
    R, rem = s1 // 32, s1 % 32           # s1 = 32*R + 4*sg + sf
    sg, sf = rem // 4, rem % 4
    return ((sg * 2 + R) * 2 + L) * 4 + sf


def assemble_output(results, n_cores=N_CORES):
    out = np.zeros((n_cores * S, 10), np.float32)
    cols = np.array([_core_sample(c) for c in range(S)])
    for c in range(n_cores):
        yc = np.asarray(results[c]['y'])
        out[c * S + cols, :] = yc.T
    return out


def kernel(**inputs) -> np.ndarray:
    from concourse.bass_utils import run_bass_kernel_spmd
    nc = get_program()
    in_maps = make_in_maps(inputs)
    res = run_bass_kernel_spmd(nc, in_maps, list(range(N_CORES)))
    return assemble_output(res.results)


# revision 25
# speedup vs baseline: 1.6202x; 1.6202x over previous
"""Trainium2 Bass kernel for nn_CONV_tiny_add_partial_558345748883.

Network: 3x [conv5x5(pad2) -> BN -> avgpool2 -> clip01] -> conv4x4(valid) -> BN1d
Input x_in [1024, 3, 32, 32] f32; output [1024, 10] f32.

v3 strategy
-----------
Measured law: per-MATMUL cost ~34ns (sem-completion serialization) once >=7
subtiles are concurrent; per-tile stream 1 col/cycle; so minimize MM count by
maximizing N*M per MM (N<=512, M = output width).

- K-packing via phase-split activation layouts (zero-copy):
  L1: x columns phase-split host-side -> K=36 (2 dx-phases x 6dy x 3ci),
      3 taps, 8 tiles (2 row-groups x 4 col bands), M=32, N=512.
  a2/a3 stored Y-PARITY-SPLIT: partitions [lane(2) x parity(2) x ci(32)].
  The L1/L2 MMs write psum col bands = (dst-lane, out-y-parity), so the
  PSUM->SBUF evac is identity on all 128 partitions, and the next layer
  gets K=64 = (parity, ci) with tap PAIRS at a uniform free-dim offset:
  L2: 18 passes (6 dx x 3 dy-pairs), 8 tiles 64x32, N=512.
  L3: 18 passes, 4 tiles 64x64 (M=64 native), N=512.
  L4: stag [parity x 64ci]; 16 taps x 1 MM (N=128), rows alternate.
- fp8e4 for L1 input + wl1 (halves input DMA; rel err ~1.1e-2 < 2e-2).
- Input DMA: 8 chunks on sync/scalar queues only, sg0+1 first, weights after.
- Evac: Relu(x+beta) split ACT/DVE; upper clip min(.,1) strided DVE passes.
"""
import os
import sys
import numpy as np

for _p in ("/opt/trn_rl_repo", "/root/.axon_site/_ro/trn_rl_repo"):
    if os.path.isdir(_p) and _p not in sys.path:
        sys.path.append(_p)

import concourse.bass as bass
import concourse.bacc as bacc
import concourse.mybir as mybir
from concourse.tile import TileContext

EPS = 1e-5
N_CORES = 8
DT = mybir.dt.float16
FP8 = mybir.dt.float8e4
F32 = mybir.dt.float32
AF = mybir.ActivationFunctionType

S = 128


# ----------------------------------------------------------------------------
# Host-side prep
# ----------------------------------------------------------------------------

def _fold_w(w, g, b, m, v):
    inv = g / np.sqrt(v + EPS)
    Wp = np.zeros((w.shape[0], w.shape[1], 6, 6), np.float32)
    for r in (0, 1):
        for s_ in (0, 1):
            Wp[:, :, r:r + 5, s_:s_ + 5] += w
    Wp *= 0.25 * inv[:, None, None, None]
    beta = (b - m * inv).astype(np.float32)
    return Wp.astype(np.float32), beta


def host_prep_weights(inputs):
    W1, beta1 = _fold_w(inputs['w1'], inputs['g1'], inputs['b1'], inputs['m1'], inputs['v1'])
    W2, beta2 = _fold_w(inputs['w2'], inputs['g2'], inputs['b2'], inputs['m2'], inputs['v2'])
    W3, beta3 = _fold_w(inputs['w3'], inputs['g3'], inputs['b3'], inputs['m3'], inputs['v3'])
    inv4 = inputs['g4'] / np.sqrt(inputs['v4'] + EPS)
    beta4 = (inputs['b4'] - inputs['m4'] * inv4).astype(np.float32)
    W4 = (inputs['w4'] * inv4[:, None, None, None]).astype(np.float32)

    d = {}
    # L1 lhsT per tap fj (dx = 2*fj + xp): K=36 rows (xp, dy, ci), M=32 co.
    wl1 = np.zeros((36, 3 * 32), np.float32)
    for fj in range(3):
        for xp in range(2):
            blk = W1[:, :, :, 2 * fj + xp].transpose(2, 1, 0).reshape(18, 32)
            wl1[18 * xp:18 * xp + 18, 32 * fj:32 * fj + 32] = blk
    w1r = np.zeros((128, 96), np.float32)
    w1r[0:36] = wl1
    w1r[64:100] = wl1
    d['wl1'] = w1r.astype(mybir.dt.np(FP8))
    # L2 lhsT per pass t2 = j*6+f (dy pair e = 2j+q): K=64 rows (q, ci), M=32.
    wl2 = np.zeros((64, 18 * 32), np.float32)
    for j in range(3):
        for f in range(6):
            t2 = j * 6 + f
            for q in range(2):
                wl2[32 * q:32 * q + 32, 32 * t2:32 * t2 + 32] = \
                    W2[:, :, 2 * j + q, f].T
    w2r = np.zeros((128, 576), np.float32)
    w2r[0:64] = wl2
    w2r[64:128] = wl2
    d['wl2'] = w2r.astype(np.float16)
    # L3 lhsT per pass: K=64 (q, ci), M=64 co.
    wl3 = np.zeros((64, 18 * 64), np.float32)
    for j in range(3):
        for f in range(6):
            t2 = j * 6 + f
            for q in range(2):
                wl3[32 * q:32 * q + 32, 64 * t2:64 * t2 + 64] = \
                    W3[:, :, 2 * j + q, f].T
    w3r = np.zeros((128, 1152), np.float32)
    w3r[0:64] = wl3
    w3r[64:128] = wl3
    d['wl3'] = w3r.astype(np.float16)
    # L4 lhsT per tap t = u*4+v: K=64 ci, M=10; row half = u parity.
    wl4 = W4.transpose(1, 2, 3, 0).reshape(64, 16 * 10)
    w4r = np.zeros((128, 160), np.float32)
    w4r[0:64] = wl4
    w4r[64:128] = wl4
    d['wl4'] = w4r.astype(np.float16)

    # bias columns: 0: A2S*beta1 (L1 evac, scaled psum)  3: beta2
    # 5: beta3   6: beta4
    bt = np.zeros((128, 8), np.float32)
    bt[:, 0] = np.tile(beta1, 4)
    bt[:, 3] = np.tile(beta2, 4)
    bt[:, 5] = np.tile(beta3, 2)
    bt[0:10, 6] = beta4
    d['betas'] = bt
    return d


def host_prep_x(x_core):
    """[128, 3, 32, 32] -> x_l1 [2 R, 36, 4 cp, 4608] fp8.

    Partition k = 18*xp + 3*dy + ci (K=36).  Free, per chunk-pair cp
    (2 sgs): [sg2, j2, y16, xh18, sf4].
    x_l1[R, k, cp, ...] = xpad[n, ci, 2y+dy, 2xh+xp],
      n = ((sg*2 + R)*2 + j)*4 + sf,  sg = 2*cp + sg2.
    """
    xp_ = np.zeros((128, 3, 36, 36), np.float32)
    xp_[:, :, 2:34, 2:34] = x_core
    # [n, dy, ci, y, x] stride-2 rows
    arr = np.stack([xp_[:, :, dy:dy + 32:2, :] for dy in range(6)], axis=1)
    # x phase split -> [n, xp, dy, ci, y, xh]
    arr = arr.reshape(128, 6, 3, 16, 18, 2).transpose(0, 5, 1, 2, 3, 4)
    arr = arr.reshape(128, 36, 16, 18)          # [n, k, y, xh]
    # n = ((sg*2+R)*2+j)*4+sf -> [sg8, R2, j2, sf4]
    arr = arr.reshape(8, 2, 2, 4, 36, 16, 18)   # [sg, R, j, sf, k, y, xh]
    arr = arr.reshape(4, 2, 2, 2, 4, 36, 16, 18)  # [cp, sg2, R, j, sf, k, y, xh]
    out = arr.transpose(2, 5, 0, 1, 3, 6, 7, 4)   # [R, k, cp, sg2, j, y, xh, sf]
    out = out.reshape(2, 36, 4, 4608)
    return np.ascontiguousarray(out).astype(mybir.dt.np(FP8))


# ----------------------------------------------------------------------------
# Bass program
# ----------------------------------------------------------------------------

def build_program():
    nc = bacc.Bacc(target_bir_lowering=False)

    x_l1 = nc.dram_tensor("x_l1", [2, 36, 4, 4608], FP8, kind="ExternalInput")
    wl1 = nc.dram_tensor("wl1", [128, 96], FP8, kind="ExternalInput")
    wl2 = nc.dram_tensor("wl2", [128, 576], DT, kind="ExternalInput")
    wl3 = nc.dram_tensor("wl3", [128, 1152], DT, kind="ExternalInput")
    wl4 = nc.dram_tensor("wl4", [128, 160], DT, kind="ExternalInput")
    betas = nc.dram_tensor("betas", [128, 8], F32, kind="ExternalInput")
    y = nc.dram_tensor("y", [10, 128], F32, kind="ExternalOutput")

    with TileContext(nc) as tc:
        with tc.tile_pool(name="consts", bufs=1) as cpool:
            wl1_t = cpool.tile([128, 96], FP8, name="wl1_t")
            wl2_t = cpool.tile([128, 576], DT, name="wl2_t")
            wl3_t = cpool.tile([128, 1152], DT, name="wl3_t")
            wl4_t = cpool.tile([128, 160], DT, name="wl4_t")
            betas_t = cpool.tile([128, 8], F32, name="betas_t")
            scr = cpool.tile([128, 512], DT, name="scr")
            nc.sync.dma_start(wl1_t[:, :], wl1.ap())
            nc.sync.dma_start(betas_t[:, :], betas.ap())
            nc.scalar.dma_start(wl2_t[:, :], wl2.ap())
            nc.scalar.dma_start(wl3_t[:, :], wl3.ap())
            nc.scalar.dma_start(wl4_t[:, :], wl4.ap())
            nc.vector.memset(scr[:, :], 0.0)

            def flush_weights():
                pass

            # a2: [lane2 x par2 x ci32][yh10, x20, s64]; a3: [yh6, x12, s64]
            a2 = cpool.tile([128, 10 * 20 * 64], DT, name="a2")
            a3 = cpool.tile([128, 6 * 12 * 64], DT, name="a3")
            stag = cpool.tile([128, 8 * 128], DT, name="stag")  # [par x ci][px8, s128]
            out_sb = cpool.tile([128, 128], F32, name="out_sb")

            a2v = a2.rearrange("p (y x s) -> p y x s", y=10, x=20)
            a3v = a3.rearrange("p (y x s) -> p y x s", y=6, x=12)
            stagv = stag.rearrange("p (t s) -> p t s", t=8)

            def memset_borders(t_, YH, W_, ns):
                # phase-plane pad: first+last yh row, and 1 col each side
                v = t_[:, :]
                nc.vector.memset(
                    bass.AP(v.tensor, v.offset,
                            [v.ap[0], [(YH - 1) * W_ * ns, 2], [1, W_ * ns]]), 0.0)
                nc.vector.memset(
                    bass.AP(v.tensor, v.offset + W_ * ns,
                            [v.ap[0], [W_ * ns, YH - 2], [(W_ - 2) * ns, 2], [1, 2 * ns]]),
                    0.0)

            memset_borders(a2, 10, 20, 64)
            memset_borders(a3, 6, 12, 64)

            def evac(engine_is_act, dst, src_ap, bias_ap):
                if engine_is_act:
                    nc.scalar.activation(dst, src_ap, AF.Relu,
                                         bias=bias_ap, scale=1.0)
                else:
                    nc.vector.tensor_scalar(
                        dst, src_ap, bias_ap, 0.0,
                        mybir.AluOpType.add, mybir.AluOpType.max)

            with (
                tc.tile_pool(name="l1io", bufs=3) as l1pool,
                tc.tile_pool(name="ps", bufs=8, space="PSUM") as pspool,
            ):
                # ---- HAM warmup: dummy matmuls fill the input-DMA window ----
                wps = pspool.tile([128, 512], F32, name="wps", tag="ps")
                for wi in range(8):
                    nc.tensor.matmul(
                        wps[0:32, :], scr[0:32, 0:32], scr[0:32, 0:512],
                        start=True, stop=True, skip_group_check=True,
                        tile_position=(0, 0),
                    )
                nc.vector.tensor_scalar_min(scr[0:1, 0:1], wps[0:1, 0:1], 1e30)

                # ================= L1 =================
                # chunk-pair cp covers sgs {2cp, 2cp+1}
                for cp in range(4):
                    xt = l1pool.tile([128, 4608], FP8, name="xt", tag="xt")
                    xv = xt.rearrange("p (g j y xh s) -> p g j y xh s",
                                      g=2, j=2, y=16, xh=18)
                    for R in range(2):
                        nc.sync.dma_start(xv[64 * R:64 * R + 36],
                                          x_l1.ap()[R, :, cp])
                    for g in range(2):
                        sg = 2 * cp + g
                        pl1 = [pspool.tile([128, 512], F32, name=f"ps1_{R}", tag="ps")
                               for R in range(2)]
                        for fj in range(3):
                            for R in range(2):
                                lhsT = wl1_t[64 * R:64 * R + 36,
                                             32 * fj:32 * fj + 32]
                                for j in range(2):
                                    for p in range(2):
                                        rhs = xv[64 * R:64 * R + 36, g, j,
                                                 p:16:2, fj:fj + 16, :]
                                        nc.tensor.matmul(
                                            pl1[R][64 * j + 32 * p:64 * j + 32 * p + 32, :],
                                            lhsT, rhs,
                                            start=(fj == 0), stop=(fj == 2),
                                            skip_group_check=True,
                                            tile_position=(64 * R, 64 * j + 32 * p),
                                        )
                        # psum [128 = (j,p) x 32co][y8, x16, sf4]
                        # slots: s1 = 32*R + 4*sg + sf
                        for R in range(2):
                            src = pl1[R][:, :].rearrange(
                                "p (y x s) -> p y x s", y=8, x=16)
                            dst = a2v[:, 1:9, 2:18, 32 * R + 4 * sg:
                                      32 * R + 4 * sg + 4]
                            use_act = (R == 0) or sg >= 5
                            if use_act:
                                nc.scalar.activation(dst, src, AF.Relu,
                                                     bias=betas_t[:, 0:1],
                                                     scale=1.0)
                            else:
                                nc.vector.tensor_scalar(
                                    dst, src, betas_t[:, 0:1], 0.0,
                                    mybir.AluOpType.add, mybir.AluOpType.max)
                            nc.vector.tensor_scalar_min(dst, dst, 1.0)

                # ================= L2 =================
                # passes t2 = j*6+f: K=64 (parity q, ci); out-y parity p' banded
                pl2 = [pspool.tile([128, 512], F32, name=f"ps2_{k}", tag="ps")
                       for k in range(4)]  # k = 2*L + r2
                for t2 in range(18):
                    j, f = t2 // 6, t2 % 6
                    for L in range(2):
                        lhsT = wl2_t[64 * L:64 * L + 64, 32 * t2:32 * t2 + 32]
                        for r2 in range(2):
                            for jp in range(2):  # dst lane j'
                                for p in range(2):  # out-y parity
                                    rhs = a2v[64 * L:64 * L + 64,
                                              j + p:j + p + 7:2, f:f + 15:2,
                                              32 * r2 + 16 * jp:32 * r2 + 16 * jp + 16]
                                    nc.tensor.matmul(
                                        pl2[2 * L + r2][64 * jp + 32 * p:
                                                        64 * jp + 32 * p + 32, :],
                                        lhsT, rhs,
                                        start=(t2 == 0), stop=(t2 == 17),
                                        skip_group_check=True,
                                        tile_position=(64 * L, 64 * jp + 32 * p),
                                    )
                for k in range(4):
                    L, r2 = k // 2, k % 2
                    src = pl2[k][:, :].rearrange(
                        "p (y x s) -> p y x s", y=4, x=8)
                    sl = 16 * (2 * L + r2)
                    dst = a3v[:, 1:5, 2:10, sl:sl + 16]
                    evac(k % 2 == 0, dst, src, betas_t[:, 3:4])
                    nc.vector.tensor_scalar_min(dst, dst, 1.0)

                # ================= L3 =================
                # 4 tiles: rows = a3 lane L', cols = out-y parity band p''
                pl3 = [pspool.tile([128, 512], F32, name=f"ps3_{k}", tag="ps")
                       for k in range(2)]  # k = L'
                for t2 in range(18):
                    j, f = t2 // 6, t2 % 6
                    for Lp in range(2):
                        lhsT = wl3_t[64 * Lp:64 * Lp + 64, 64 * t2:64 * t2 + 64]
                        for p in range(2):
                            rhs = a3v[64 * Lp:64 * Lp + 64,
                                      j + p:j + p + 3:2, f:f + 7:2, :]
                            nc.tensor.matmul(
                                pl3[Lp][64 * p:64 * p + 64, :], lhsT, rhs,
                                start=(t2 == 0), stop=(t2 == 17),
                                skip_group_check=True,
                                tile_position=(64 * Lp, 64 * p),
                            )
                # evac: psum [128 = par x 64co][y2, x4, s64] -> stag px = y*4+x
                for Lp in range(2):
                    src = pl3[Lp][:, :].rearrange(
                        "p (y x s) -> p (y x) s", y=2, x=4)
                    dst = stagv[:, :, 64 * Lp:64 * Lp + 64]
                    evac(Lp == 0, dst, src, betas_t[:, 5:6])
                    nc.vector.tensor_scalar_min(dst, dst, 1.0)

                # ================= L4 =================
                # separate psum per row-parity stream (avoid concurrent
                # accumulation races into one region), then add at evac
                ps4 = [pspool.tile([128, 128], F32, name=f"ps4_{q}", tag="ps")
                       for q in range(2)]
                nseen = [0, 0]
                for t in range(16):
                    u, v = t // 4, t % 4
                    q = u % 2
                    lhsT = wl4_t[64 * q:64 * q + 64, 10 * t:10 * t + 10]
                    rhs = stagv[64 * q:64 * q + 64, (u // 2) * 4 + v, :]
                    nc.tensor.matmul(
                        ps4[q][0:10, :], lhsT, rhs,
                        start=(nseen[q] == 0), stop=(nseen[q] == 7),
                        skip_group_check=True,
                        tile_position=(64 * q, 0),
                    )
                    nseen[q] += 1
                nc.scalar.activation(
                    out_sb[0:10, :], ps4[0][0:10, :],
                    AF.Identity, bias=betas_t[0:10, 6:7], scale=1.0,
                )
                nc.vector.tensor_tensor(
                    out_sb[0:10, :], ps4[1][0:10, :], out_sb[0:10, :],
                    mybir.AluOpType.add)
                nc.sync.dma_start(y.ap(), out_sb[0:10, :])

        return nc


_NC_CACHE = None


def get_program():
    global _NC_CACHE
    if _NC_CACHE is None:
        nc = build_program()
        if not nc.is_finalized():
            nc.finalize()
        _NC_CACHE = nc
    return _NC_CACHE


def make_in_maps(inputs, n_cores=N_CORES):
    wdict = host_prep_weights(inputs)
    in_maps = []
    for c in range(n_cores):
        x_core = np.asarray(inputs['x_in'][c * S:(c + 1) * S], np.float32)
        m = {'x_l1': host_prep_x(x_core)}
        m.update(wdict)
        in_maps.append(m)
    return in_maps


def _core_sample(col):
    """Output column (0..127) -> per-core sample index n."""
    Lp, t = col // 64, col % 64          # stag: s3 = 64*L' + slot2
    half, k = t // 16, t % 16            # slot2 = 16*(2L + r2) + k
    L, r2 = half // 2, half % 2
    jp = Lp                              # dst a3 lane = j'
    s1 = 32 * r2 + 16 * jp + k           # a2 slot of lane L
    R, rem = s1 // 32, s1 % 32           # s1 = 32*R + 4*sg + sf
    sg, sf = rem // 4, rem % 4
    return ((sg * 2 + R) * 2 + L) * 4 + sf


def assemble_output(results, n_cores=N_CORES):
    out = np.zeros((n_cores * S, 10), np.float32)
    cols = np.array([_core_sample(c) for c in range(S)])
    for c in range(n_cores):
        yc = np.asarray(results[c]['y'])
        out[c * S + cols, :] = yc.T
    return out


def kernel(**inputs) -> np.ndarray:
    from concourse.bass_utils import run_bass_kernel_spmd
    nc = get_program()
    in_maps = make_in_maps(inputs)
    res = run_bass_kernel_spmd(nc, in_maps, list(range(N_CORES)))
    return assemble_output(res.results)


# revision 27
# speedup vs baseline: 1.6233x; 1.0019x over previous
"""Trainium2 Bass kernel for nn_CONV_tiny_add_partial_558345748883.

Network: 3x [conv5x5(pad2) -> BN -> avgpool2 -> clip01] -> conv4x4(valid) -> BN1d
Input x_in [1024, 3, 32, 32] f32; output [1024, 10] f32.

v3 strategy
-----------
Measured law: per-MATMUL cost ~34ns (sem-completion serialization) once >=7
subtiles are concurrent; per-tile stream 1 col/cycle; so minimize MM count by
maximizing N*M per MM (N<=512, M = output width).

- K-packing via phase-split activation layouts (zero-copy):
  L1: x columns phase-split host-side -> K=36 (2 dx-phases x 6dy x 3ci),
      3 taps, 8 tiles (2 row-groups x 4 col bands), M=32, N=512.
  a2/a3 stored Y-PARITY-SPLIT: partitions [lane(2) x parity(2) x ci(32)].
  The L1/L2 MMs write psum col bands = (dst-lane, out-y-parity), so the
  PSUM->SBUF evac is identity on all 128 partitions, and the next layer
  gets K=64 = (parity, ci) with tap PAIRS at a uniform free-dim offset:
  L2: 18 passes (6 dx x 3 dy-pairs), 8 tiles 64x32, N=512.
  L3: 18 passes, 4 tiles 64x64 (M=64 native), N=512.
  L4: stag [parity x 64ci]; 16 taps x 1 MM (N=128), rows alternate.
- fp8e4 for L1 input + wl1 (halves input DMA; rel err ~1.1e-2 < 2e-2).
- Input DMA: 8 chunks on sync/scalar queues only, sg0+1 first, weights after.
- Evac: Relu(x+beta) split ACT/DVE; upper clip min(.,1) strided DVE passes.
"""
import os
import sys
import numpy as np

for _p in ("/opt/trn_rl_repo", "/root/.axon_site/_ro/trn_rl_repo"):
    if os.path.isdir(_p) and _p not in sys.path:
        sys.path.append(_p)

import concourse.bass as bass
import concourse.bacc as bacc
import concourse.mybir as mybir
from concourse.tile import TileContext

EPS = 1e-5
N_CORES = 8
DT = mybir.dt.float16
FP8 = mybir.dt.float8e4
F32 = mybir.dt.float32
AF = mybir.ActivationFunctionType

S = 128


# ----------------------------------------------------------------------------
# Host-side prep
# ----------------------------------------------------------------------------

def _fold_w(w, g, b, m, v):
    inv = g / np.sqrt(v + EPS)
    Wp = np.zeros((w.shape[0], w.shape[1], 6, 6), np.float32)
    for r in (0, 1):
        for s_ in (0, 1):
            Wp[:, :, r:r + 5, s_:s_ + 5] += w
    Wp *= 0.25 * inv[:, None, None, None]
    beta = (b - m * inv).astype(np.float32)
    return Wp.astype(np.float32), beta


def host_prep_weights(inputs):
    W1, beta1 = _fold_w(inputs['w1'], inputs['g1'], inputs['b1'], inputs['m1'], inputs['v1'])
    W2, beta2 = _fold_w(inputs['w2'], inputs['g2'], inputs['b2'], inputs['m2'], inputs['v2'])
    W3, beta3 = _fold_w(inputs['w3'], inputs['g3'], inputs['b3'], inputs['m3'], inputs['v3'])
    inv4 = inputs['g4'] / np.sqrt(inputs['v4'] + EPS)
    beta4 = (inputs['b4'] - inputs['m4'] * inv4).astype(np.float32)
    W4 = (inputs['w4'] * inv4[:, None, None, None]).astype(np.float32)

    d = {}
    # L1 lhsT per tap fj (dx = 2*fj + xp): K=36 rows (xp, dy, ci), M=32 co.
    wl1 = np.zeros((36, 3 * 32), np.float32)
    for fj in range(3):
        for xp in range(2):
            blk = W1[:, :, :, 2 * fj + xp].transpose(2, 1, 0).reshape(18, 32)
            wl1[18 * xp:18 * xp + 18, 32 * fj:32 * fj + 32] = blk
    w1r = np.zeros((128, 96), np.float32)
    w1r[0:36] = wl1
    w1r[64:100] = wl1
    d['wl1'] = w1r.astype(mybir.dt.np(FP8))
    # L2 lhsT per pass t2 = j*6+f (dy pair e = 2j+q): K=64 rows (q, ci), M=32.
    wl2 = np.zeros((64, 18 * 32), np.float32)
    for j in range(3):
        for f in range(6):
            t2 = j * 6 + f
            for q in range(2):
                wl2[32 * q:32 * q + 32, 32 * t2:32 * t2 + 32] = \
                    W2[:, :, 2 * j + q, f].T
    w2r = np.zeros((128, 576), np.float32)
    w2r[0:64] = wl2
    w2r[64:128] = wl2
    d['wl2'] = w2r.astype(np.float16)
    # L3 lhsT per pass: K=64 (q, ci), M=64 co.
    wl3 = np.zeros((64, 18 * 64), np.float32)
    for j in range(3):
        for f in range(6):
            t2 = j * 6 + f
            for q in range(2):
                wl3[32 * q:32 * q + 32, 64 * t2:64 * t2 + 64] = \
                    W3[:, :, 2 * j + q, f].T
    w3r = np.zeros((128, 1152), np.float32)
    w3r[0:64] = wl3
    w3r[64:128] = wl3
    d['wl3'] = w3r.astype(np.float16)
    # L4 lhsT per tap t = u*4+v: K=64 ci, M=10; row half = u parity.
    wl4 = W4.transpose(1, 2, 3, 0).reshape(64, 16 * 10)
    w4r = np.zeros((128, 160), np.float32)
    w4r[0:64] = wl4
    w4r[64:128] = wl4
    d['wl4'] = w4r.astype(np.float16)

    # bias columns: 0: A2S*beta1 (L1 evac, scaled psum)  3: beta2
    # 5: beta3   6: beta4
    bt = np.zeros((128, 8), np.float32)
    bt[:, 0] = np.tile(beta1, 4)
    bt[:, 3] = np.tile(beta2, 4)
    bt[:, 5] = np.tile(beta3, 2)
    bt[0:10, 6] = beta4
    d['betas'] = bt
    return d


def host_prep_x(x_core):
    """[128, 3, 32, 32] -> x_l1 [2 R, 36, 4 cp, 4608] fp8.

    Partition k = 18*xp + 3*dy + ci (K=36).  Free, per chunk-pair cp
    (2 sgs): [sg2, j2, y16, xh18, sf4].
    x_l1[R, k, cp, ...] = xpad[n, ci, 2y+dy, 2xh+xp],
      n = ((sg*2 + R)*2 + j)*4 + sf,  sg = 2*cp + sg2.
    """
    xp_ = np.zeros((128, 3, 36, 36), np.float32)
    xp_[:, :, 2:34, 2:34] = x_core
    # [n, dy, ci, y, x] stride-2 rows
    arr = np.stack([xp_[:, :, dy:dy + 32:2, :] for dy in range(6)], axis=1)
    # x phase split -> [n, xp, dy, ci, y, xh]
    arr = arr.reshape(128, 6, 3, 16, 18, 2).transpose(0, 5, 1, 2, 3, 4)
    arr = arr.reshape(128, 36, 16, 18)          # [n, k, y, xh]
    # n = ((sg*2+R)*2+j)*4+sf -> [sg8, R2, j2, sf4]
    arr = arr.reshape(8, 2, 2, 4, 36, 16, 18)   # [sg, R, j, sf, k, y, xh]
    arr = arr.reshape(4, 2, 2, 2, 4, 36, 16, 18)  # [cp, sg2, R, j, sf, k, y, xh]
    out = arr.transpose(2, 5, 0, 1, 3, 6, 7, 4)   # [R, k, cp, sg2, j, y, xh, sf]
    out = out.reshape(2, 36, 4, 4608)
    return np.ascontiguousarray(out).astype(mybir.dt.np(FP8))


# ----------------------------------------------------------------------------
# Bass program
# ----------------------------------------------------------------------------

def build_program():
    nc = bacc.Bacc(target_bir_lowering=False)

    x_l1 = nc.dram_tensor("x_l1", [2, 36, 4, 4608], FP8, kind="ExternalInput")
    wl1 = nc.dram_tensor("wl1", [128, 96], FP8, kind="ExternalInput")
    wl2 = nc.dram_tensor("wl2", [128, 576], DT, kind="ExternalInput")
    wl3 = nc.dram_tensor("wl3", [128, 1152], DT, kind="ExternalInput")
    wl4 = nc.dram_tensor("wl4", [128, 160], DT, kind="ExternalInput")
    betas = nc.dram_tensor("betas", [128, 8], F32, kind="ExternalInput")
    y = nc.dram_tensor("y", [10, 128], F32, kind="ExternalOutput")

    with TileContext(nc) as tc:
        with tc.tile_pool(name="consts", bufs=1) as cpool:
            wl1_t = cpool.tile([128, 96], FP8, name="wl1_t")
            wl2_t = cpool.tile([128, 576], DT, name="wl2_t")
            wl3_t = cpool.tile([128, 1152], DT, name="wl3_t")
            wl4_t = cpool.tile([128, 160], DT, name="wl4_t")
            betas_t = cpool.tile([128, 8], F32, name="betas_t")
            scr = cpool.tile([128, 512], DT, name="scr")
            nc.sync.dma_start(wl1_t[:, :], wl1.ap())
            nc.sync.dma_start(betas_t[:, :], betas.ap())
            nc.scalar.dma_start(wl2_t[:, :], wl2.ap())
            nc.scalar.dma_start(wl3_t[:, :], wl3.ap())
            nc.scalar.dma_start(wl4_t[:, :], wl4.ap())
            nc.vector.memset(scr[:, :], 0.0)

            def flush_weights():
                pass

            # a2: [lane2 x par2 x ci32][yh10, x20, s64]; a3: [yh6, x12, s64]
            a2 = cpool.tile([128, 10 * 20 * 64], DT, name="a2")
            a3 = cpool.tile([128, 6 * 12 * 64], DT, name="a3")
            stag = cpool.tile([128, 8 * 128], DT, name="stag")  # [par x ci][px8, s128]
            out_sb = cpool.tile([128, 128], F32, name="out_sb")

            a2v = a2.rearrange("p (y x s) -> p y x s", y=10, x=20)
            a3v = a3.rearrange("p (y x s) -> p y x s", y=6, x=12)
            stagv = stag.rearrange("p (t s) -> p t s", t=8)

            def memset_borders(t_, YH, W_, ns):
                # phase-plane pad: first+last yh row, and 1 col each side
                v = t_[:, :]
                nc.vector.memset(
                    bass.AP(v.tensor, v.offset,
                            [v.ap[0], [(YH - 1) * W_ * ns, 2], [1, W_ * ns]]), 0.0)
                nc.vector.memset(
                    bass.AP(v.tensor, v.offset + W_ * ns,
                            [v.ap[0], [W_ * ns, YH - 2], [(W_ - 2) * ns, 2], [1, 2 * ns]]),
                    0.0)

            memset_borders(a2, 10, 20, 64)
            memset_borders(a3, 6, 12, 64)

            def evac(engine_is_act, dst, src_ap, bias_ap):
                if engine_is_act:
                    nc.scalar.activation(dst, src_ap, AF.Relu,
                                         bias=bias_ap, scale=1.0)
                else:
                    nc.vector.tensor_scalar(
                        dst, src_ap, bias_ap, 0.0,
                        mybir.AluOpType.add, mybir.AluOpType.max)

            with (
                tc.tile_pool(name="l1io", bufs=3) as l1pool,
                tc.tile_pool(name="ps", bufs=8, space="PSUM") as pspool,
            ):
                # ---- HAM warmup: back-to-back accumulating dummy matmuls
                # fill the input-DMA window so the PE clock is at 8/8 when
                # real work starts ----
                NWARM = 9
                wps = pspool.tile([128, 512], F32, name="wps", tag="ps")
                for wi in range(NWARM):
                    nc.tensor.matmul(
                        wps[0:32, :], scr[0:32, 0:32], scr[0:32, 0:512],
                        start=(wi == 0), stop=(wi == NWARM - 1),
                        skip_group_check=True,
                        tile_position=(0, 0),
                    )
                nc.vector.tensor_scalar_min(scr[0:1, 0:1], wps[0:1, 0:1], 1e30)

                # ================= L1 =================
                # chunk-pair cp covers sgs {2cp, 2cp+1}
                for cp in range(4):
                    xt = l1pool.tile([128, 4608], FP8, name="xt", tag="xt")
                    xv = xt.rearrange("p (g j y xh s) -> p g j y xh s",
                                      g=2, j=2, y=16, xh=18)
                    for R in range(2):
                        eng = nc.sync if R == 0 else nc.scalar
                        eng.dma_start(xv[64 * R:64 * R + 36],
                                      x_l1.ap()[R, :, cp])
                    for g in range(2):
                        sg = 2 * cp + g
                        pl1 = [pspool.tile([128, 512], F32, name=f"ps1_{R}", tag="ps")
                               for R in range(2)]
                        for fj in range(3):
                            for R in range(2):
                                lhsT = wl1_t[64 * R:64 * R + 36,
                                             32 * fj:32 * fj + 32]
                                for j in range(2):
                                    for p in range(2):
                                        rhs = xv[64 * R:64 * R + 36, g, j,
                                                 p:16:2, fj:fj + 16, :]
                                        nc.tensor.matmul(
                                            pl1[R][64 * j + 32 * p:64 * j + 32 * p + 32, :],
                                            lhsT, rhs,
                                            start=(fj == 0), stop=(fj == 2),
                                            skip_group_check=True,
                                            tile_position=(64 * R, 64 * j + 32 * p),
                                        )
                        # psum [128 = (j,p) x 32co][y8, x16, sf4]
                        # slots: s1 = 32*R + 4*sg + sf
                        for R in range(2):
                            src = pl1[R][:, :].rearrange(
                                "p (y x s) -> p y x s", y=8, x=16)
                            dst = a2v[:, 1:9, 2:18, 32 * R + 4 * sg:
                                      32 * R + 4 * sg + 4]
                            use_act = (R == 0) or sg >= 5
                            if use_act:
                                nc.scalar.activation(dst, src, AF.Relu,
                                                     bias=betas_t[:, 0:1],
                                                     scale=1.0)
                            else:
                                nc.vector.tensor_scalar(
                                    dst, src, betas_t[:, 0:1], 0.0,
                                    mybir.AluOpType.add, mybir.AluOpType.max)
                            nc.vector.tensor_scalar_min(dst, dst, 1.0)

                # ================= L2 =================
                # passes t2 = j*6+f: K=64 (parity q, ci); out-y parity p' banded
                pl2 = [pspool.tile([128, 512], F32, name=f"ps2_{k}", tag="ps")
                       for k in range(4)]  # k = 2*L + r2
                for t2 in range(18):
                    j, f = t2 // 6, t2 % 6
                    for L in range(2):
                        lhsT = wl2_t[64 * L:64 * L + 64, 32 * t2:32 * t2 + 32]
                        for r2 in range(2):
                            for jp in range(2):  # dst lane j'
                                for p in range(2):  # out-y parity
                                    rhs = a2v[64 * L:64 * L + 64,
                                              j + p:j + p + 7:2, f:f + 15:2,
                                              32 * r2 + 16 * jp:32 * r2 + 16 * jp + 16]
                                    nc.tensor.matmul(
                                        pl2[2 * L + r2][64 * jp + 32 * p:
                                                        64 * jp + 32 * p + 32, :],
                                        lhsT, rhs,
                                        start=(t2 == 0), stop=(t2 == 17),
                                        skip_group_check=True,
                                        tile_position=(64 * L, 64 * jp + 32 * p),
                                    )
                for k in range(4):
                    L, r2 = k // 2, k % 2
                    src = pl2[k][:, :].rearrange(
                        "p (y x s) -> p y x s", y=4, x=8)
                    sl = 16 * (2 * L + r2)
                    dst = a3v[:, 1:5, 2:10, sl:sl + 16]
                    evac(k % 2 == 0, dst, src, betas_t[:, 3:4])
                    nc.vector.tensor_scalar_min(dst, dst, 1.0)

                # ================= L3 =================
                # 4 tiles: rows = a3 lane L', cols = out-y parity band p''
                pl3 = [pspool.tile([128, 512], F32, name=f"ps3_{k}", tag="ps")
                       for k in range(2)]  # k = L'
                for t2 in range(18):
                    j, f = t2 // 6, t2 % 6
                    for Lp in range(2):
                        lhsT = wl3_t[64 * Lp:64 * Lp + 64, 64 * t2:64 * t2 + 64]
                        for p in range(2):
                            rhs = a3v[64 * Lp:64 * Lp + 64,
                                      j + p:j + p + 3:2, f:f + 7:2, :]
                            nc.tensor.matmul(
                                pl3[Lp][64 * p:64 * p + 64, :], lhsT, rhs,
                                start=(t2 == 0), stop=(t2 == 17),
                                skip_group_check=True,
                                tile_position=(64 * Lp, 64 * p),
                            )
                # evac: psum [128 = par x 64co][y2, x4, s64] -> stag px = y*4+x
                for Lp in range(2):
                    src = pl3[Lp][:, :].rearrange(
                        "p (y x s) -> p (y x) s", y=2, x=4)
                    dst = stagv[:, :, 64 * Lp:64 * Lp + 64]
                    evac(Lp == 0, dst, src, betas_t[:, 5:6])
                    nc.vector.tensor_scalar_min(dst, dst, 1.0)

                # ================= L4 =================
                # separate psum per row-parity stream (avoid concurrent
                # accumulation races into one region), then add at evac
                ps4 = [pspool.tile([128, 128], F32, name=f"ps4_{q}", tag="ps")
                       for q in range(2)]
                nseen = [0, 0]
                for t in range(16):
                    u, v = t // 4, t % 4
                    q = u % 2
                    lhsT = wl4_t[64 * q:64 * q + 64, 10 * t:10 * t + 10]
                    rhs = stagv[64 * q:64 * q + 64, (u // 2) * 4 + v, :]
                    nc.tensor.matmul(
                        ps4[q][0:10, :], lhsT, rhs,
                        start=(nseen[q] == 0), stop=(nseen[q] == 7),
                        skip_group_check=True,
                        tile_position=(64 * q, 0),
                    )
                    nseen[q] += 1
                nc.scalar.activation(
                    out_sb[0:10, :], ps4[0][0:10, :],
                    AF.Identity, bias=betas_t[0:10, 6:7], scale=1.0,
                )
                nc.vector.tensor_tensor(
                    out_sb[0:10, :], ps4[1][0:10, :], out_sb[0:10, :],
                    mybir.AluOpType.add)
                nc.sync.dma_start(y.ap(), out_sb[0:10, :])

        return nc


_NC_CACHE = None


def get_program():
    global _NC_CACHE
    if _NC_CACHE is None:
        nc = build_program()
        if not nc.is_finalized():
            nc.finalize()
        _NC_CACHE = nc
    return _NC_CACHE


def make_in_maps(inputs, n_cores=N_CORES):
    wdict = host_prep_weights(inputs)
    in_maps = []
    for c in range(n_cores):
        x_core = np.asarray(inputs['x_in'][c * S:(c + 1) * S], np.float32)
        m = {'x_l1': host_prep_x(x_core)}
        m.update(wdict)
        in_maps.append(m)
    return in_maps


def _core_sample(col):
    """Output column (0..127) -> per-core sample index n."""
    Lp, t = col // 64, col % 64          # stag: s3 = 64*L' + slot2
    half, k = t // 16, t % 16            # slot2 = 16*(2L + r2) + k
    L, r2 = half // 2, half % 2
    jp = Lp                              # dst a3 lane = j'
    s1 = 32 * r2 + 16 * jp + k           # a2 slot of lane L
    R, rem = s1 // 32, s1 % 32           # s1 = 32*R + 4*sg + sf
    sg, sf = rem // 4, rem % 4
    return ((sg * 2 + R) * 2 + L) * 4 + sf


def assemble_output(results, n_cores=N_CORES):
    out = np.zeros((n_cores * S, 10), np.float32)
    cols = np.array([_core_sample(c) for c in range(S)])
    for c in range(n_cores):
        yc = np.asarray(results[c]['y'])
        out[c * S + cols, :] = yc.T
    return out


def kernel(**inputs) -> np.ndarray:
    from concourse.bass_utils import run_bass_kernel_spmd
    nc = get_program()
    in_maps = make_in_maps(inputs)
    res = run_bass_kernel_spmd(nc, in_maps, list(range(N_CORES)))
    return assemble_output(res.results)


# revision 30
# speedup vs baseline: 1.6628x; 1.0243x over previous
"""Trainium2 Bass kernel for nn_CONV_tiny_add_partial_558345748883.

Network: 3x [conv5x5(pad2) -> BN -> avgpool2 -> clip01] -> conv4x4(valid) -> BN1d
Input x_in [1024, 3, 32, 32] f32; output [1024, 10] f32.

v3 strategy
-----------
Measured law: per-MATMUL cost ~34ns (sem-completion serialization) once >=7
subtiles are concurrent; per-tile stream 1 col/cycle; so minimize MM count by
maximizing N*M per MM (N<=512, M = output width).

- K-packing via phase-split activation layouts (zero-copy):
  L1: x columns phase-split host-side -> K=36 (2 dx-phases x 6dy x 3ci),
      3 taps, 8 tiles (2 row-groups x 4 col bands), M=32, N=512.
  a2/a3 stored Y-PARITY-SPLIT: partitions [lane(2) x parity(2) x ci(32)].
  The L1/L2 MMs write psum col bands = (dst-lane, out-y-parity), so the
  PSUM->SBUF evac is identity on all 128 partitions, and the next layer
  gets K=64 = (parity, ci) with tap PAIRS at a uniform free-dim offset:
  L2: 18 passes (6 dx x 3 dy-pairs), 8 tiles 64x32, N=512.
  L3: 18 passes, 4 tiles 64x64 (M=64 native), N=512.
  L4: stag [parity x 64ci]; 16 taps x 1 MM (N=128), rows alternate.
- fp8e4 for L1 input + wl1 (halves input DMA; rel err ~1.1e-2 < 2e-2).
- Input DMA: 8 chunks on sync/scalar queues only, sg0+1 first, weights after.
- Evac: Relu(x+beta) split ACT/DVE; upper clip min(.,1) strided DVE passes.
"""
import os
import sys
import numpy as np

for _p in ("/opt/trn_rl_repo", "/root/.axon_site/_ro/trn_rl_repo"):
    if os.path.isdir(_p) and _p not in sys.path:
        sys.path.append(_p)

import concourse.bass as bass
import concourse.bacc as bacc
import concourse.mybir as mybir
from concourse.tile import TileContext

EPS = 1e-5
N_CORES = 8
DT = mybir.dt.float16
FP8 = mybir.dt.float8e4
F32 = mybir.dt.float32
AF = mybir.ActivationFunctionType

S = 128


# ----------------------------------------------------------------------------
# Host-side prep
# ----------------------------------------------------------------------------

def _fold_w(w, g, b, m, v):
    inv = g / np.sqrt(v + EPS)
    Wp = np.zeros((w.shape[0], w.shape[1], 6, 6), np.float32)
    for r in (0, 1):
        for s_ in (0, 1):
            Wp[:, :, r:r + 5, s_:s_ + 5] += w
    Wp *= 0.25 * inv[:, None, None, None]
    beta = (b - m * inv).astype(np.float32)
    return Wp.astype(np.float32), beta


def host_prep_weights(inputs):
    W1, beta1 = _fold_w(inputs['w1'], inputs['g1'], inputs['b1'], inputs['m1'], inputs['v1'])
    W2, beta2 = _fold_w(inputs['w2'], inputs['g2'], inputs['b2'], inputs['m2'], inputs['v2'])
    W3, beta3 = _fold_w(inputs['w3'], inputs['g3'], inputs['b3'], inputs['m3'], inputs['v3'])
    inv4 = inputs['g4'] / np.sqrt(inputs['v4'] + EPS)
    beta4 = (inputs['b4'] - inputs['m4'] * inv4).astype(np.float32)
    W4 = (inputs['w4'] * inv4[:, None, None, None]).astype(np.float32)

    d = {}
    # L1 lhsT per tap fj (dx = 2*fj + xp): K=36 rows (xp, dy, ci), M=32 co.
    wl1 = np.zeros((36, 3 * 32), np.float32)
    for fj in range(3):
        for xp in range(2):
            blk = W1[:, :, :, 2 * fj + xp].transpose(2, 1, 0).reshape(18, 32)
            wl1[18 * xp:18 * xp + 18, 32 * fj:32 * fj + 32] = blk
    w1r = np.zeros((128, 96), np.float32)
    w1r[0:36] = wl1
    w1r[64:100] = wl1
    d['wl1'] = w1r.astype(mybir.dt.np(FP8))
    # L2 lhsT per pass t2 = j*6+f (dy pair e = 2j+q): K=64 rows (q, ci), M=32.
    wl2 = np.zeros((64, 18 * 32), np.float32)
    for j in range(3):
        for f in range(6):
            t2 = j * 6 + f
            for q in range(2):
                wl2[32 * q:32 * q + 32, 32 * t2:32 * t2 + 32] = \
                    W2[:, :, 2 * j + q, f].T
    w2r = np.zeros((128, 576), np.float32)
    w2r[0:64] = wl2
    w2r[64:128] = wl2
    d['wl2'] = w2r.astype(np.float16)
    # L3 lhsT per pass: K=64 (q, ci), M=64 co.
    wl3 = np.zeros((64, 18 * 64), np.float32)
    for j in range(3):
        for f in range(6):
            t2 = j * 6 + f
            for q in range(2):
                wl3[32 * q:32 * q + 32, 64 * t2:64 * t2 + 64] = \
                    W3[:, :, 2 * j + q, f].T
    w3r = np.zeros((128, 1152), np.float32)
    w3r[0:64] = wl3
    w3r[64:128] = wl3
    d['wl3'] = w3r.astype(np.float16)
    # L4 lhsT per tap t = u*4+v: K=64 ci, M=10; row half = u parity.
    wl4 = W4.transpose(1, 2, 3, 0).reshape(64, 16 * 10)
    w4r = np.zeros((128, 160), np.float32)
    w4r[0:64] = wl4
    w4r[64:128] = wl4
    d['wl4'] = w4r.astype(np.float16)

    # bias columns: 0: A2S*beta1 (L1 evac, scaled psum)  3: beta2
    # 5: beta3   6: beta4
    bt = np.zeros((128, 8), np.float32)
    bt[:, 0] = np.tile(beta1, 4)
    bt[:, 3] = np.tile(beta2, 4)
    bt[:, 5] = np.tile(beta3, 2)
    bt[0:10, 6] = beta4
    d['betas'] = bt
    return d


def host_prep_x(x_core):
    """[128, 3, 32, 32] -> x_l1 [2 R, 36, 4 cp, 4608] fp8.

    Partition k = 18*xp + 3*dy + ci (K=36).  Free, per chunk-pair cp
    (2 sgs): [sg2, j2, y16, xh18, sf4].
    x_l1[R, k, cp, ...] = xpad[n, ci, 2y+dy, 2xh+xp],
      n = ((sg*2 + R)*2 + j)*4 + sf,  sg = 2*cp + sg2.
    """
    xp_ = np.zeros((128, 3, 36, 36), np.float32)
    xp_[:, :, 2:34, 2:34] = x_core
    # [n, dy, ci, y, x] stride-2 rows
    arr = np.stack([xp_[:, :, dy:dy + 32:2, :] for dy in range(6)], axis=1)
    # x phase split -> [n, xp, dy, ci, y, xh]
    arr = arr.reshape(128, 6, 3, 16, 18, 2).transpose(0, 5, 1, 2, 3, 4)
    arr = arr.reshape(128, 36, 16, 18)          # [n, k, y, xh]
    # n = ((sg*2+R)*2+j)*4+sf -> [sg8, R2, j2, sf4]
    arr = arr.reshape(8, 2, 2, 4, 36, 16, 18)   # [sg, R, j, sf, k, y, xh]
    arr = arr.reshape(4, 2, 2, 2, 4, 36, 16, 18)  # [cp, sg2, R, j, sf, k, y, xh]
    out = arr.transpose(2, 5, 0, 1, 3, 6, 7, 4)   # [R, k, cp, sg2, j, y, xh, sf]
    out = out.reshape(2, 36, 4, 4608)
    return np.ascontiguousarray(out).astype(mybir.dt.np(FP8))


# ----------------------------------------------------------------------------
# Bass program
# ----------------------------------------------------------------------------

def build_program():
    nc = bacc.Bacc(target_bir_lowering=False)

    x_l1 = nc.dram_tensor("x_l1", [2, 36, 4, 4608], FP8, kind="ExternalInput")
    wl1 = nc.dram_tensor("wl1", [128, 96], FP8, kind="ExternalInput")
    wl2 = nc.dram_tensor("wl2", [128, 576], DT, kind="ExternalInput")
    wl3 = nc.dram_tensor("wl3", [128, 1152], DT, kind="ExternalInput")
    wl4 = nc.dram_tensor("wl4", [128, 160], DT, kind="ExternalInput")
    betas = nc.dram_tensor("betas", [128, 8], F32, kind="ExternalInput")
    y = nc.dram_tensor("y", [10, 128], F32, kind="ExternalOutput")

    with TileContext(nc) as tc:
        with tc.tile_pool(name="consts", bufs=1) as cpool:
            wl1_t = cpool.tile([128, 96], FP8, name="wl1_t")
            wl2_t = cpool.tile([128, 576], DT, name="wl2_t")
            wl3_t = cpool.tile([128, 1152], DT, name="wl3_t")
            wl4_t = cpool.tile([128, 160], DT, name="wl4_t")
            betas_t = cpool.tile([128, 8], F32, name="betas_t")
            scr = cpool.tile([128, 512], DT, name="scr")
            nc.sync.dma_start(wl1_t[:, :], wl1.ap())
            nc.sync.dma_start(betas_t[:, :], betas.ap())
            nc.vector.memset(scr[:, :], 0.0)
            deferred_w = [1]

            def flush_weights():
                if not deferred_w:
                    return
                nc.scalar.dma_start(wl2_t[:, :], wl2.ap())
                nc.scalar.dma_start(wl3_t[:, :], wl3.ap())
                nc.scalar.dma_start(wl4_t[:, :], wl4.ap())
                deferred_w.clear()

            # a2: [lane2 x par2 x ci32][yh10, x20, s64]; a3: [yh6, x12, s64]
            a2 = cpool.tile([128, 10 * 20 * 64], DT, name="a2")
            a3 = cpool.tile([128, 6 * 12 * 64], DT, name="a3")
            stag = cpool.tile([128, 8 * 128], DT, name="stag")  # [par x ci][px8, s128]
            out_sb = cpool.tile([128, 128], F32, name="out_sb")

            a2v = a2.rearrange("p (y x s) -> p y x s", y=10, x=20)
            a3v = a3.rearrange("p (y x s) -> p y x s", y=6, x=12)
            stagv = stag.rearrange("p (t s) -> p t s", t=8)

            def memset_borders(t_, YH, W_, ns):
                # phase-plane pad: first+last yh row, and 1 col each side
                v = t_[:, :]
                nc.vector.memset(
                    bass.AP(v.tensor, v.offset,
                            [v.ap[0], [(YH - 1) * W_ * ns, 2], [1, W_ * ns]]), 0.0)
                nc.vector.memset(
                    bass.AP(v.tensor, v.offset + W_ * ns,
                            [v.ap[0], [W_ * ns, YH - 2], [(W_ - 2) * ns, 2], [1, 2 * ns]]),
                    0.0)

            memset_borders(a2, 10, 20, 64)
            memset_borders(a3, 6, 12, 64)

            def evac(engine_is_act, dst, src_ap, bias_ap):
                if engine_is_act:
                    nc.scalar.activation(dst, src_ap, AF.Relu,
                                         bias=bias_ap, scale=1.0)
                else:
                    nc.vector.tensor_scalar(
                        dst, src_ap, bias_ap, 0.0,
                        mybir.AluOpType.add, mybir.AluOpType.max)

            with (
                tc.tile_pool(name="l1io", bufs=3) as l1pool,
                tc.tile_pool(name="ps", bufs=8, space="PSUM") as pspool,
            ):
                # ---- HAM warmup: back-to-back accumulating dummy matmuls
                # fill the input-DMA window so the PE clock is at 8/8 when
                # real work starts ----
                NWARM = 9
                wps = pspool.tile([128, 512], F32, name="wps", tag="ps")
                for wi in range(NWARM):
                    nc.tensor.matmul(
                        wps[0:128, :], scr[0:128, 0:128], scr[0:128, 0:512],
                        start=(wi == 0), stop=(wi == NWARM - 1),
                        skip_group_check=True,
                        tile_position=(0, 0),
                    )
                nc.vector.tensor_scalar_min(scr[0:1, 0:1], wps[0:1, 0:1], 1e30)

                # ================= L1 =================
                # chunk-pair cp covers sgs {2cp, 2cp+1}
                for cp in range(4):
                    xt = l1pool.tile([128, 4608], FP8, name="xt", tag="xt")
                    xv = xt.rearrange("p (g j y xh s) -> p g j y xh s",
                                      g=2, j=2, y=16, xh=18)
                    for R in range(2):
                        eng = nc.sync if R == 0 else nc.scalar
                        eng.dma_start(xv[64 * R:64 * R + 36],
                                      x_l1.ap()[R, :, cp])
                    if cp == 1:
                        flush_weights()
                    for g in range(2):
                        sg = 2 * cp + g
                        pl1 = [pspool.tile([128, 512], F32, name=f"ps1_{R}", tag="ps")
                               for R in range(2)]
                        for fj in range(3):
                            for R in range(2):
                                lhsT = wl1_t[64 * R:64 * R + 36,
                                             32 * fj:32 * fj + 32]
                                for j in range(2):
                                    for p in range(2):
                                        rhs = xv[64 * R:64 * R + 36, g, j,
                                                 p:16:2, fj:fj + 16, :]
                                        nc.tensor.matmul(
                                            pl1[R][64 * j + 32 * p:64 * j + 32 * p + 32, :],
                                            lhsT, rhs,
                                            start=(fj == 0), stop=(fj == 2),
                                            skip_group_check=True,
                                            tile_position=(64 * R, 64 * j + 32 * p),
                                        )
                        # psum [128 = (j,p) x 32co][y8, x16, sf4]
                        # slots: s1 = 32*R + 4*sg + sf
                        for R in range(2):
                            src = pl1[R][:, :].rearrange(
                                "p (y x s) -> p y x s", y=8, x=16)
                            dst = a2v[:, 1:9, 2:18, 32 * R + 4 * sg:
                                      32 * R + 4 * sg + 4]
                            use_act = (R == 0) or sg >= 5
                            if use_act:
                                nc.scalar.activation(dst, src, AF.Relu,
                                                     bias=betas_t[:, 0:1],
                                                     scale=1.0)
                            else:
                                nc.vector.tensor_scalar(
                                    dst, src, betas_t[:, 0:1], 0.0,
                                    mybir.AluOpType.add, mybir.AluOpType.max)
                            nc.vector.tensor_scalar_min(dst, dst, 1.0)

                # ================= L2 =================
                # passes t2 = j*6+f: K=64 (parity q, ci); out-y parity p' banded
                pl2 = [pspool.tile([128, 512], F32, name=f"ps2_{k}", tag="ps")
                       for k in range(4)]  # k = 2*L + r2
                for t2 in range(18):
                    j, f = t2 // 6, t2 % 6
                    for L in range(2):
                        lhsT = wl2_t[64 * L:64 * L + 64, 32 * t2:32 * t2 + 32]
                        for r2 in range(2):
                            for jp in range(2):  # dst lane j'
                                for p in range(2):  # out-y parity
                                    rhs = a2v[64 * L:64 * L + 64,
                                              j + p:j + p + 7:2, f:f + 15:2,
                                              32 * r2 + 16 * jp:32 * r2 + 16 * jp + 16]
                                    nc.tensor.matmul(
                                        pl2[2 * L + r2][64 * jp + 32 * p:
                                                        64 * jp + 32 * p + 32, :],
                                        lhsT, rhs,
                                        start=(t2 == 0), stop=(t2 == 17),
                                        skip_group_check=True,
                                        tile_position=(64 * L, 64 * jp + 32 * p),
                                    )
                for k in range(4):
                    L, r2 = k // 2, k % 2
                    src = pl2[k][:, :].rearrange(
                        "p (y x s) -> p y x s", y=4, x=8)
                    sl = 16 * (2 * L + r2)
                    dst = a3v[:, 1:5, 2:10, sl:sl + 16]
                    evac(k % 2 == 0, dst, src, betas_t[:, 3:4])
                    nc.vector.tensor_scalar_min(dst, dst, 1.0)

                # ================= L3 =================
                # 4 tiles: rows = a3 lane L', cols = out-y parity band p''
                pl3 = [pspool.tile([128, 512], F32, name=f"ps3_{k}", tag="ps")
                       for k in range(2)]  # k = L'
                for t2 in range(18):
                    j, f = t2 // 6, t2 % 6
                    for Lp in range(2):
                        lhsT = wl3_t[64 * Lp:64 * Lp + 64, 64 * t2:64 * t2 + 64]
                        for p in range(2):
                            rhs = a3v[64 * Lp:64 * Lp + 64,
                                      j + p:j + p + 3:2, f:f + 7:2, :]
                            nc.tensor.matmul(
                                pl3[Lp][64 * p:64 * p + 64, :], lhsT, rhs,
                                start=(t2 == 0), stop=(t2 == 17),
                                skip_group_check=True,
                                tile_position=(64 * Lp, 64 * p),
                            )
                # evac: psum [128 = par x 64co][y2, x4, s64] -> stag px = y*4+x
                for Lp in range(2):
                    src = pl3[Lp][:, :].rearrange(
                        "p (y x s) -> p (y x) s", y=2, x=4)
                    dst = stagv[:, :, 64 * Lp:64 * Lp + 64]
                    evac(Lp == 0, dst, src, betas_t[:, 5:6])
                    nc.vector.tensor_scalar_min(dst, dst, 1.0)

                # ================= L4 =================
                # separate psum per row-parity stream (avoid concurrent
                # accumulation races into one region), then add at evac
                ps4 = [pspool.tile([128, 128], F32, name=f"ps4_{q}", tag="ps")
                       for q in range(2)]
                nseen = [0, 0]
                for t in range(16):
                    u, v = t // 4, t % 4
                    q = u % 2
                    lhsT = wl4_t[64 * q:64 * q + 64, 10 * t:10 * t + 10]
                    rhs = stagv[64 * q:64 * q + 64, (u // 2) * 4 + v, :]
                    nc.tensor.matmul(
                        ps4[q][0:10, :], lhsT, rhs,
                        start=(nseen[q] == 0), stop=(nseen[q] == 7),
                        skip_group_check=True,
                        tile_position=(64 * q, 0),
                    )
                    nseen[q] += 1
                nc.scalar.activation(
                    out_sb[0:10, :], ps4[0][0:10, :],
                    AF.Identity, bias=betas_t[0:10, 6:7], scale=1.0,
                )
                nc.vector.tensor_tensor(
                    out_sb[0:10, :], ps4[1][0:10, :], out_sb[0:10, :],
                    mybir.AluOpType.add)
                nc.sync.dma_start(y.ap(), out_sb[0:10, :])

        return nc


_NC_CACHE = None


def get_program():
    global _NC_CACHE
    if _NC_CACHE is None:
        nc = build_program()
        if not nc.is_finalized():
            nc.finalize()
        _NC_CACHE = nc
    return _NC_CACHE


def make_in_maps(inputs, n_cores=N_CORES):
    wdict = host_prep_weights(inputs)
    in_maps = []
    for c in range(n_cores):
        x_core = np.asarray(inputs['x_in'][c * S:(c + 1) * S], np.float32)
        m = {'x_l1': host_prep_x(x_core)}
        m.update(wdict)
        in_maps.append(m)
    return in_maps


def _core_sample(col):
    """Output column (0..127) -> per-core sample index n."""
    Lp, t = col // 64, col % 64          # stag: s3 = 64*L' + slot2
    half, k = t // 16, t % 16            # slot2 = 16*(2L + r2) + k
    L, r2 = half // 2, half % 2
    jp = Lp                              # dst a3 lane = j'
    s1 = 32 * r2 + 16 * jp + k           # a2 slot of lane L
    R, rem = s1 // 32, s1 % 32           # s1 = 32*R + 4*sg + sf
    sg, sf = rem // 4, rem % 4
    return ((sg * 2 + R) * 2 + L) * 4 + sf


def assemble_output(results, n_cores=N_CORES):
    out = np.zeros((n_cores * S, 10), np.float32)
    cols = np.array([_core_sample(c) for c in range(S)])
    for c in range(n_cores):
        yc = np.asarray(results[c]['y'])
        out[c * S + cols, :] = yc.T
    return out


def kernel(**inputs) -> np.ndarray:
    from concourse.bass_utils import run_bass_kernel_spmd
    nc = get_program()
    in_maps = make_in_maps(inputs)
    res = run_bass_kernel_spmd(nc, in_maps, list(range(N_CORES)))
    return assemble_output(res.results)
